# revision 15
# baseline (speedup 1.0000x reference)
"""Trainium2 Bass kernel for nn_CLIP topk_masking (v3: block-aligned text DMA).

Computes, for full inputs (self-contained; shapes hardcoded):
    probability = image_features @ ima_proto.T          # [B, NP]
    thr_r       = k-th largest of probability row r
    sel[r, j]   = probability[r, j] >= thr_r            # top-k prototype mask
    text_n      = exp(logit_scale) * text_raw / ||text_raw||_row
    logits[r,c] = (image_features @ text_n.T)[r,c] * sel[r, c // 10]

Sharding: data-parallel over the batch axis across 8 NeuronCores
(rows 512/core); prototypes and text features replicated.

v3 layout: text is loaded in chunks of 1250 classes as [125p, 10t, 512d]
with partition p holding rows 10p..10p+9 (one 20 KB contiguous HBM run per
partition), so the load DMA is HBM-bound instead of descriptor-rate bound.
Each PE-transposed tile t yields logit columns for classes {10p + t}, i.e.
one class BLOCK per partition index -- the top-k mask for the whole chunk is
just sel[:, 125c : 125(c+1)] broadcast over t, and the (t,p) -> class
reorder is done with zero-cost access patterns on the mask-apply.
"""

import os
from contextlib import ExitStack

import numpy as np

import concourse.bass as bass
import concourse.tile as tile
from concourse import bacc, mybir
from concourse.bass_utils import run_bass_kernel_spmd

# Problem shapes (hardcoded per contract).
B, D, NP, NC, CPT = 4096, 512, 1000, 10000, 10
NCORES = 8
RLOC = B // NCORES          # 512 rows per core
RT = RLOC // 128            # 4 row tiles per core
KD = D // 128               # 4 contraction chunks
CT = 125                    # classes-per-tile / partitions used for text & proto
TT = CPT                    # text rows per partition slab (= class block size)
CCH = CT * TT               # classes per chunk = 1250
NCH = NC // CCH             # 8 chunks
MMH = 625                   # matmul N split (1250 = 2*625; bf16 moving max 1024)
PREF = 3                    # chunks prefetched at kernel start
NEG = -1.0e30

F32 = mybir.dt.float32
BF16 = mybir.dt.bfloat16

# Byte-reduction knobs: keep text in HBM as bf16 (host converts once) and/or
# store the masked logits as bf16 (host widens back to f32 on gather).
# Numerically: bf16 text adds ~2e-3 rel err, bf16 output ~4e-3 -- both small
# against the 2e-2 gate.
TEXT_BF16 = bool(int(os.environ.get("K_TEXT_BF16", "0")))
OUT_BF16 = bool(int(os.environ.get("K_OUT_BF16", "0")))
TDT = BF16 if TEXT_BF16 else F32
ODT = BF16 if OUT_BF16 else F32

LAST_RESULTS = None


def _emit(ctx: ExitStack, tc, img, proto, text, out, k: int, inv_s2: float):
    nc = tc.nc
    AF = mybir.ActivationFunctionType
    OP = mybir.AluOpType

    const = ctx.enter_context(tc.tile_pool(name="const", bufs=1))
    persist = ctx.enter_context(tc.tile_pool(name="persist", bufs=1))

    # Identity for PE transposes comes in from the host: building it with
    # gpsimd.affine_select stalls the whole SWDGE queue ~11us behind a
    # one-time ucode IRAM upload.
    identd = tc.nc.dram_tensor("ident", [128, 128], F32, kind="ExternalInput").ap()
    ident = const.tile([128, 128], F32)
    nc.sync.dma_start(ident[:], identd)
    identb = const.tile([128, 128], BF16)
    nc.vector.tensor_copy(identb[:], ident[:])


    # imgT[p, kc, r] = img[r, kc*128 + p]; sel[rt][p, j] = top-k mask row 128*rt+p.
    imgT = persist.tile([128, KD, RLOC], F32)
    imgTb = persist.tile([128, KD, RLOC], BF16)
    sels = []

    # Text chunk loads: 2.56 MB each, one 20 KB contiguous descriptor per
    # partition -> HBM-rate on the sync HWDGE queue. (The casting SWDGE
    # variant only sustains ~110 GB/s read-side; f32 loads + an engine-side
    # bf16 cast are faster overall.)
    pb_traw = ctx.enter_context(tc.tile_pool(name="pb_traw", bufs=3))
    # PSUM pools are shared between phase A and phase B (a phase-A-only pool
    # would hold its banks until release, serializing chunk 0's transposes
    # and matmuls behind the last probability matmul).
    pb_psT = ctx.enter_context(tc.tile_pool(name="pb_psT", bufs=2, space="PSUM"))
    pb_psM = ctx.enter_context(tc.tile_pool(name="pb_psM", bufs=2, space="PSUM"))
    traw_tiles = {}

    # Measured on this part: one queue sustains ~320 GB/s and the
    # read-on-sync + write-on-gpsimd mix ~376 GB/s -- PROVIDED descriptors
    # are large. The 3D "(p t) d" access pattern lowers to 2 KB descriptors
    # (t and d unmerged) and craters to ~125 GB/s, so the (t d) merge below
    # is load-bearing: one contiguous 20 KB descriptor per partition.
    def load_chunk(c: int):
        t_ = pb_traw.tile([CT, TT, D], TDT, name=f"traw{c}", tag="traw")
        dst = t_[:].rearrange("p t d -> p (t d)")
        src = text[c * CCH:(c + 1) * CCH].rearrange("(p t) d -> p (t d)", t=TT)
        # Two half-chunk DMAs (10 KB descriptors) instead of one 2.56 MB
        # transfer: more DMAs in flight on the queue keeps more SDMA
        # engines busy (a single large DMA measured only ~110 GB/s).
        h = TT * D // 2
        nc.sync.dma_start(dst[:, :h], src[:, :h])
        nc.sync.dma_start(dst[:, h:], src[:, h:])
        traw_tiles[c] = t_

    # ---------- Phase A: img/proto transpose, probability matmul, top-k ----------
    with (
        tc.tile_pool(name="pa_sb", bufs=1) as pa_sb,
        tc.tile_pool(name="pa_work", bufs=1) as pa_work,
    ):
        # A tiny dummy SWDGE transfer first: it absorbs the gpsimd DMA
        # path's one-time ~11us ucode/IRAM setup before the stores ride it.
        swdge_warm = pa_sb.tile([128, 128], F32)
        nc.gpsimd.dma_start(swdge_warm[:], identd)
        # proto loads contiguously on sync (16 KB descriptors): partition p
        # slab t holds proto row 8p+t. The permutation is undone when sel
        # is written (the top-k threshold itself is order-independent).
        proto_sb = pa_sb.tile([CT, NP // CT, D], F32)
        nc.sync.dma_start(proto_sb[:].rearrange("p t d -> p (t d)"),
                          proto.rearrange("(p t) d -> p (t d)", t=NP // CT))
        # img loads contiguously (8 KB descriptors): partition p, slab t
        # holds image row 4p+t. The row permutation flows through imgT /
        # sel / pm consistently and is undone by the store access pattern.
        img_sb = pa_sb.tile([128, RT, D], F32)
        nc.sync.dma_start(img_sb[:].rearrange("p t d -> p (t d)"),
                          img.rearrange("(p t) d -> p (t d)", t=RT))
        for c0 in range(PREF):
            load_chunk(c0)

        # PE warmup: ~3.5us of dummy transposes so the HAM clock gate flips
        # to full rate before the real work lands.
        warm = pb_psT.tile([128, KD, 128], F32, tag="pt")
        for _ in range(16):
            nc.tensor.transpose(warm[:, 0, :], ident[:], ident[:])

        for rt in range(RT):
            for kc in range(KD):
                pi = pb_psT.tile([128, KD, 128], F32, tag="pt")
                nc.tensor.transpose(
                    pi[:, 0, :], img_sb[:, rt, kc * 128:(kc + 1) * 128], ident[:])
                nc.vector.tensor_copy(imgT[:, kc, rt * 128:(rt + 1) * 128],
                                      pi[:, 0, :])
        # bf16 copy for the logit matmul.
        nc.vector.tensor_copy(imgTb[:], imgT[:])

        protoT = pa_sb.tile([128, KD, NP], F32)
        for t in range(NP // CT):
            pp = pb_psT.tile([128, KD, 128], F32, tag="pt")
            for kc in range(KD):
                nc.tensor.transpose(
                    pp[:, kc, :CT], proto_sb[:, t, kc * 128:(kc + 1) * 128],
                    ident[:CT, :CT])
            nc.vector.tensor_copy(protoT[:, :, t * CT:(t + 1) * CT],
                                  pp[:, :, :CT])

        for rt in range(RT):
            pprt = pb_psM.tile([128, 3, 512], F32, tag="pm")
            ppr = pprt[:, :2, :]
            for h in range(2):
                for kc in range(KD):
                    # fp32 (not bf16): ranking precision decides the mask.
                    nc.tensor.matmul(
                        ppr[:, h, :NP // 2],
                        imgT[:, kc, rt * 128:(rt + 1) * 128],
                        protoT[:, kc, h * (NP // 2):(h + 1) * (NP // 2)],
                        start=(kc == 0), stop=(kc == KD - 1),
                    )
            prob = pa_work.tile([128, NP], F32, tag="prob")
            nc.vector.tensor_copy(
                prob[:].rearrange("p (a b) -> p a b", a=2), ppr[:, :, :NP // 2])
            m8a = pa_work.tile([128, 8], F32, tag="m8a")
            nc.vector.max(m8a[:], prob[:])
            if k <= 8:
                thr = m8a[:, k - 1:k]
            else:
                repl = pa_work.tile([128, NP], F32, tag="repl")
                nc.vector.match_replace(repl[:], m8a[:], prob[:], NEG)
                m8b = pa_work.tile([128, 8], F32, tag="m8b")
                nc.vector.max(m8b[:], repl[:])
                thr = m8b[:, k - 9:k - 8]
            sel = persist.tile([128, NP], F32, tag=f"sel{rt}")
            # prob column t*125+p is proto row 8p+t -> scatter back to true
            # proto order through strided views (pure AP, no extra pass).
            NPT = NP // CT
            nc.vector.tensor_scalar(
                sel[:].rearrange("P (p t) -> P t p", t=NPT),
                prob[:].rearrange("P (t p) -> P t p", p=CT),
                thr, None, op0=OP.is_ge)
            sels.append(sel)

    # ---------- Phase B: text normalize+transpose, logit matmul, mask, store ----------
    with (
        tc.tile_pool(name="pb_sq", bufs=3) as pb_sq,
        tc.tile_pool(name="pb_nrm", bufs=4) as pb_nrm,
        tc.tile_pool(name="pb_sc", bufs=4) as pb_sc,
        tc.tile_pool(name="pb_ttT", bufs=3) as pb_ttT,
        tc.tile_pool(name="pb_stage", bufs=2) as pb_stage,
    ):
        for c in range(NCH):
            if c + PREF < NCH:
                load_chunk(c + PREF)
            traw = traw_tiles[c]

            # Row norms^2 on the scalar engine (tensor_tensor_reduce on DVE
            # does not execute on this HW runtime), ||t||/s via sqrt there
            # too, reciprocal on vector (tiny).
            nrm = pb_nrm.tile([CT, TT], F32, tag="nrm")
            for t in range(TT):
                sq = pb_sq.tile([CT, D], BF16, tag="sq")
                nc.scalar.activation(
                    sq[:], traw[:, t], AF.Square, accum_out=nrm[:, t:t + 1])
            nrs = pb_nrm.tile([CT, TT], F32, tag="nrs")
            # sqrt(||t||^2 * exp(-2*logit_scale)) = ||t|| / s
            nc.scalar.activation(nrs[:], nrm[:], AF.Sqrt, scale=inv_s2)
            rcp = pb_nrm.tile([CT, TT], F32, tag="rcp")
            nc.vector.reciprocal(rcp[:], nrs[:])       # s / ||t||

            # ttT[:, kc, t*125 + p] = class (10p + t) of this chunk, bf16.
            ttT = pb_ttT.tile([128, KD, CCH], BF16)
            for t in range(TT):
                # normalize-scale with f32->bf16 downcast fused in (vector)
                sc = pb_sc.tile([CT, D], BF16, tag="sc")
                nc.vector.tensor_scalar(
                    sc[:], traw[:, t], rcp[:, t:t + 1], None, op0=OP.mult)
                # inner dim padded 125->128 so each kc slice lands 4B-aligned
                # in PSUM (hard requirement for matmul-class instructions).
                pt = pb_psT.tile([128, KD, 128], BF16, tag="pt")
                for kc in range(KD):
                    nc.tensor.transpose(
                        pt[:, kc, :CT], sc[:, kc * 128:(kc + 1) * 128],
                        identb[:CT, :CT])
                # PSUM -> SBUF drain, split between vector (bf16 2x rate)
                # and scalar to balance the two engines.
                if t % 2 == 0:
                    nc.vector.tensor_copy(ttT[:, :, t * CT:(t + 1) * CT],
                                          pt[:, :, :CT])
                else:
                    nc.scalar.copy(ttT[:, :, t * CT:(t + 1) * CT],
                                   pt[:, :, :CT])

            for rt in range(RT):
                # A matmul output may not cross a PSUM bank (512 f32), so
                # the 1250 columns go out as 512+512+226 segments.
                pm = pb_psM.tile([128, 3, 512], F32, name=f"pmc{c}r{rt}", tag="pm")
                for h in range(3):
                    nh = min(512, CCH - 512 * h)
                    for kc in range(KD):
                        nc.tensor.matmul(
                            pm[:, h, :nh],
                            imgTb[:, kc, rt * 128:(rt + 1) * 128],
                            ttT[:, kc, 512 * h:512 * h + nh],
                            start=(kc == 0), stop=(kc == KD - 1),
                        )
                stage = pb_stage.tile([128, CCH], ODT, name=f"stg{rt}",
                                      tag=f"stg{rt}")
                selb = sels[rt][:, c * CT:(c + 1) * CT]
                selb = selb.broadcast_to([128, CT, TT])
                pmf = pm[:].rearrange("P a b -> P (a b)")[:, :CCH]
                # stage column (10p + t) <- pm column (t*125 + p) * sel[p]
                nc.vector.tensor_tensor(
                    stage[:].rearrange("P (p t) -> P p t", t=TT),
                    pmf.rearrange("P (t p) -> P p t", p=CT),
                    selb, op=OP.mult)
                # partition p of row-tile rt holds image row 4p+rt
                dst_ap = out.rearrange("(p t) c -> t p c", t=RT)[
                    rt, :, c * CCH:(c + 1) * CCH]
                if c == NCH - 1 and rt >= 2:
                    # Final chunk: text loads are done, split the tail
                    # stores across both queues.
                    nc.sync.dma_start(dst_ap, stage[:])
                else:
                    # Stores ride the gpsimd SWDGE queue so they never
                    # head-of-line block the text loads on sync.
                    nc.gpsimd.dma_start(dst_ap, stage[:])


def _build(k: int, inv_s2: float):
    nc = bacc.Bacc("TRN2", target_bir_lowering=False, debug=False)
    img = nc.dram_tensor("img", [RLOC, D], F32, kind="ExternalInput").ap()
    proto = nc.dram_tensor("proto", [NP, D], F32, kind="ExternalInput").ap()
    text = nc.dram_tensor("text", [NC, D], TDT, kind="ExternalInput").ap()
    out = nc.dram_tensor("out", [RLOC, NC], ODT, kind="ExternalOutput").ap()
    with tile.TileContext(nc) as tc:
        with ExitStack() as ctx:
            _emit(ctx, tc, img, proto, text, out, k, inv_s2)
    nc.compile()
    return nc


def kernel(image_features, ima_proto, text_features_raw, logit_scale, num_test):
    global LAST_RESULTS
    img = np.ascontiguousarray(np.asarray(image_features, dtype=np.float32))
    proto = np.ascontiguousarray(np.asarray(ima_proto, dtype=np.float32))
    text = np.ascontiguousarray(np.asarray(text_features_raw, dtype=np.float32))
    assert img.shape == (B, D) and proto.shape == (NP, D) and text.shape == (NC, D)
    s = float(np.asarray(logit_scale))
    k = min(int(np.asarray(num_test)), NP)
    assert 1 <= k <= 16, f"kernel supports k in [1, 16], got {k}"
    inv_s2 = float(np.exp(-2.0 * s))

    nc = _build(k, inv_s2)
    ident = np.eye(128, dtype=np.float32)
    if TEXT_BF16:
        import ml_dtypes
        text = text.astype(ml_dtypes.bfloat16)
    in_maps = [
        {"img": img[i * RLOC:(i + 1) * RLOC], "proto": proto, "text": text,
         "ident": ident}
        for i in range(NCORES)
    ]
    trace = bool(int(os.environ.get("BASS_KERNEL_TRACE", "0")))
    res = run_bass_kernel_spmd(nc, in_maps, list(range(NCORES)), trace=trace)
    LAST_RESULTS = res
    full = np.concatenate([r["out"] for r in res.results], axis=0)
    return np.ascontiguousarray(full.astype(np.float32))
